# revision 70
# baseline (speedup 1.0000x reference)
"""Causal self-attention TRN2 kernel: 8-way head-parallel (2 heads/core).

Layout strategy (per core c, heads h0=2c, h1=2c+1):
  - Host pre-lays x -> xT3 [128, 8, 4096] bf16 (c-within-chunk, c-chunk, token)
    and weights [128, 8, 128] bf16 with q/k head dims [32 evens | 32 odds]
    (de-interleaved RoPE); cos/sin tables pre-broadcast to [128, T] bf16 with
    the RoPE sign pattern baked into sin rows.
  - QKV: q/k computed transposed ([head-dim, tok]) via bf16 matmuls over 8
    contraction chunks; V computed directly in [tok, head-dim] layout (lhsT =
    x chunk), killing the PE transposes the AV matmul would otherwise need.
  - RoPE on q/k PSUM with partition-aligned DVE ops (signed sin table +
    32-row swap via SBUF-SBUF DMA); result written bf16 into qkT.
  - V_aug [tok, V(64) | ones(64)] bf16 so the AV matmul also produces the
    softmax denominator in psum rows 64:127.
  - Scores computed TRANSPOSED: S^T[k,q] per 128-k-chunk; exp on ACT
    (scale=1/8, no max subtraction; |s|<~20); causal diagonal handled at
    128-col granularity: columns left of the diagonal chunk are never
    computed, only the 128x128 triangle piece gets a gpsimd affine_select.
  - AV accumulates per 128-col piece so unmasked pieces never wait on the
    mask; out-proj: lhsT=outT chunk [128,128] bf16, rhs=woT [128,1024] bf16.
  - Emission interleaves QKV/out-proj matmuls as fillers between attention
    chunks so PE fills the gaps while ACT runs exp.
  - Host sums 8 bf16 partials (the tensor-parallel all-reduce) and reshapes.
"""

import sys

if "/opt/trn_rl_repo" not in sys.path:
    sys.path.insert(0, "/opt/trn_rl_repo")

import numpy as np
import ml_dtypes

import concourse.bass as bass
import concourse.tile as tile
from concourse import bacc, mybir

F32 = mybir.dt.float32
BF16 = mybir.dt.bfloat16
EXP = mybir.ActivationFunctionType.Exp

B, T, D, H, DH = 2, 2048, 1024, 16, 64
NCORES = 8
TOK = B * T          # 4096
PB = 512             # qkv block (tokens)
NB = TOK // PB       # 8 blocks
QB = 512             # attention q-block
KC = 128             # k chunk
CPJ = D // 128       # 8 contraction chunks
FILL_RATE = 3        # filler PE-ops pulled per attention chunk


def build_program():
    nc = bacc.Bacc("TRN2", target_bir_lowering=False, debug=False,
                   num_devices=NCORES)
    xT3 = nc.dram_tensor("xT3", [128, CPJ, TOK], BF16, kind="ExternalInput").ap()
    wq3 = nc.dram_tensor("wq3", [128, CPJ, 128], BF16, kind="ExternalInput").ap()
    wk3 = nc.dram_tensor("wk3", [128, CPJ, 128], BF16, kind="ExternalInput").ap()
    wv3 = nc.dram_tensor("wv3", [128, CPJ, 128], BF16, kind="ExternalInput").ap()
    woT = nc.dram_tensor("woT", [128, D], BF16, kind="ExternalInput").ap()
    cosF = nc.dram_tensor("cosF", [128, T], BF16, kind="ExternalInput").ap()
    sinF = nc.dram_tensor("sinF", [128, T], BF16, kind="ExternalInput").ap()
    partial = nc.dram_tensor("partial", [TOK, D], BF16,
                             kind="ExternalOutput").ap()

    with tile.TileContext(nc) as tc:
        with tc.tile_pool(name="sb", bufs=1) as sb, \
             tc.tile_pool(name="ps", bufs=1, space="PSUM") as ps:
            # persistent SBUF tiles
            xfull = sb.tile([128, CPJ, TOK], BF16, name="xfull", tag="xfull")
            wq_sb = sb.tile([128, CPJ, 128], BF16, name="wq_sb", tag="wq_sb")
            wk_sb = sb.tile([128, CPJ, 128], BF16, name="wk_sb", tag="wk_sb")
            wv_sb = sb.tile([128, CPJ, 128], BF16, name="wv_sb", tag="wv_sb")
            woT_sb = sb.tile([128, D], BF16, name="woT_sb", tag="woT_sb")
            cos_full = sb.tile([128, T], BF16, name="cos_full", tag="cos_full")
            sin_full = sb.tile([128, T], BF16, name="sin_full", tag="sin_full")
            qkT = sb.tile([128, 2, TOK], BF16, name="qkT", tag="qkT")
            outT = sb.tile([128, TOK], BF16, name="outT", tag="outT")
            vaug = [sb.tile([128, T // KC, 128], BF16, name=f"vaug{u}",
                            tag=f"vaug{u}") for u in range(4)]

            warm = sb.tile([128, 128], BF16, name="warm", tag="warm")

            def emit_setup_dmas():
                nc.gpsimd.memset(warm[:], 0.0)
                nc.sync.dma_start(out=xfull[:, :, 0:PB], in_=xT3[:, :, 0:PB])
                nc.sync.dma_start(out=wq_sb[:], in_=wq3[:])
                nc.sync.dma_start(out=wk_sb[:], in_=wk3[:])
                nc.sync.dma_start(out=cos_full[:], in_=cosF[:])
                nc.sync.dma_start(out=sin_full[:], in_=sinF[:])
                nc.sync.dma_start(out=wv_sb[:], in_=wv3[:])
                nc.sync.dma_start(out=xfull[:, :, PB:2 * PB],
                                  in_=xT3[:, :, PB:2 * PB])
                nc.sync.dma_start(out=woT_sb[:], in_=woT[:])
                for u in range(4):
                    nc.gpsimd.memset(vaug[u][:, :, 64:128], 1.0)
                # PE warm-up during the DMA lead-in: ramps the tensor engine
                # to full p-state before the first real matmul.
                for i in range(48):
                    wp = ps.tile([128, 512], F32, name=f"warm{i}", tag="ops",
                                 bufs=2)
                    nc.tensor.matmul(wp[:, 0:128], warm[:], warm[:],
                                     start=True, stop=True)

            block_done = [False] * NB

            def gen_qkv(s):
                """QKV block s (tokens s*PB..): yields after each PE matmul."""
                scol = s * PB
                tcol = (s % (T // PB)) * PB
                b = s // (T // PB)
                if s + 2 < NB:   # just-in-time load of a later x slice
                    nxt = (s + 2) * PB
                    nc.sync.dma_start(out=xfull[:, :, nxt:nxt + PB],
                                      in_=xT3[:, :, nxt:nxt + PB])
                qp = ps.tile([128, PB], F32, name=f"qps{s}", tag="qps", bufs=1)
                kp = ps.tile([128, PB], F32, name=f"kps{s}", tag="kps", bufs=1)
                vp = ps.tile([128, 4, 128], F32, name=f"vps{s}", tag="ops",
                             bufs=2)
                for j in range(CPJ):
                    st, sp = (j == 0), (j == CPJ - 1)
                    nc.tensor.matmul(qp[:], wq_sb[:, j, :],
                                     xfull[:, j, scol:scol + PB],
                                     start=st, stop=sp)
                    yield
                    nc.tensor.matmul(kp[:], wk_sb[:, j, :],
                                     xfull[:, j, scol:scol + PB],
                                     start=st, stop=sp)
                    yield
                # RoPE: ra = raw*cos, rs = raw*sin, rw = 32-row swap of rs
                ra = sb.tile([128, 2, PB], F32, name=f"ra{s}", tag="ra", bufs=2)
                rs = sb.tile([128, 2, PB], F32, name=f"rs{s}", tag="rs", bufs=2)
                rw = sb.tile([128, 2, PB], F32, name=f"rw{s}", tag="rw", bufs=2)
                cs = cos_full[:, tcol:tcol + PB]
                sn = sin_full[:, tcol:tcol + PB]
                nc.vector.tensor_mul(ra[:, 0, :], qp[:], cs)
                nc.vector.tensor_mul(rs[:, 0, :], qp[:], sn)
                nc.vector.tensor_mul(ra[:, 1, :], kp[:], cs)
                nc.vector.tensor_mul(rs[:, 1, :], kp[:], sn)
                for blk in range(4):
                    src = (blk ^ 1) * 32
                    nc.sync.dma_start(out=rw[blk * 32:(blk + 1) * 32, :, :],
                                      in_=rs[src:src + 32, :, :])
                nc.vector.tensor_add(qkT[:, 0, scol:scol + PB], ra[:, 0, :],
                                     rw[:, 0, :])
                nc.vector.tensor_add(qkT[:, 1, scol:scol + PB], ra[:, 1, :],
                                     rw[:, 1, :])
                for j in range(CPJ):
                    for tt in range(4):
                        # one accumulation group per psum bank: single
                        # start on the first matmul, stop on the last
                        # (first touch of each byte overwrites pending-zero)
                        nc.tensor.matmul(
                            vp[:, tt, :],
                            xfull[:, j, scol + tt * 128:scol + (tt + 1) * 128],
                            wv_sb[:, j, :], start=(j == 0 and tt == 0),
                            stop=(j == CPJ - 1 and tt == 3),
                            skip_group_check=True)
                        yield
                ck0 = (s % (T // PB)) * 4
                for h in range(2):
                    nc.vector.tensor_copy(
                        vaug[b * 2 + h][:, ck0:ck0 + 4, 0:64],
                        vp[:, :, h * 64:(h + 1) * 64])
                block_done[s] = True

            # ---- filler management ----
            fillers = []

            def pull(n):
                for _ in range(n):
                    while fillers:
                        try:
                            next(fillers[0])
                            break
                        except StopIteration:
                            fillers.pop(0)
                    else:
                        return

            def need_block(sbi):
                while not block_done[sbi]:
                    if not fillers:
                        raise RuntimeError(f"no fillers but block {sbi} undone")
                    try:
                        next(fillers[0])
                    except StopIteration:
                        fillers.pop(0)

            def gen_outproj(bt, mm):
                col = bt * T + mm * 128
                ob = sb.tile([128, D], BF16, name=f"osb{bt}_{mm}", tag="osb",
                             bufs=8)
                opt0 = ps.tile([128, 512], F32, name=f"opsa{bt}_{mm}",
                               tag="ops", bufs=2)
                nc.tensor.matmul(opt0[:], outT[:, col:col + 128],
                                 woT_sb[:, 0:512], start=True, stop=True)
                yield
                opt1 = ps.tile([128, 512], F32, name=f"opsb{bt}_{mm}",
                               tag="ops", bufs=2)
                if mm % 2 == 1 or mm >= 12:
                    nc.scalar.copy(out=ob[:, 0:512], in_=opt0[:])
                else:
                    nc.vector.tensor_copy(out=ob[:, 0:512], in_=opt0[:])
                nc.tensor.matmul(opt1[:], outT[:, col:col + 128],
                                 woT_sb[:, 512:1024], start=True, stop=True)
                yield
                nc.vector.tensor_copy(out=ob[:, 512:1024], in_=opt1[:])
                nc.sync.dma_start(out=partial[col:col + 128, :], in_=ob[:])

            def emit_attn_block(u, s4):
                """Attention for unit u (batch u//2, head u%2), q-block s4."""
                b, h = u // 2, u % 2
                hr = h * 64
                tb = b * T
                qc = tb + s4 * QB
                need_block(b * 4 + s4)
                avt = ps.tile([128, QB], F32, name=f"av{u}_{s4}", tag="av",
                              bufs=1)
                njc = (s4 + 1) * 4
                pend = []           # deferred av matmuls (stagger depth 2)

                def flush(k):
                    while len(pend) > k:
                        pend.pop(0)()

                for j in range(njc):
                    di = j - s4 * 4          # >=0: diagonal chunk index
                    c0 = di * 128 if di > 0 else 0
                    kc = tb + j * KC
                    spst = ps.tile([128, QB], F32, name=f"sps{u}_{s4}_{j}",
                                   tag="sps", bufs=3)
                    nc.tensor.matmul(spst[:, c0:QB],
                                     qkT[hr:hr + 64, 1, kc:kc + KC],
                                     qkT[hr:hr + 64, 0, qc + c0:qc + QB],
                                     start=True, stop=True)
                    pTt = sb.tile([128, QB], BF16, name=f"pT{u}_{s4}_{j}",
                                  tag="pT", bufs=10)
                    nc.scalar.activation(pTt[:, c0:QB], spst[:, c0:QB], EXP,
                                         scale=0.125)
                    if di >= 0:
                        nc.gpsimd.affine_select(
                            out=pTt[:, di * 128:(di + 1) * 128],
                            in_=pTt[:, di * 128:(di + 1) * 128],
                            compare_op=mybir.AluOpType.is_ge,
                            fill=0.0, base=0, pattern=[[1, 128]],
                            channel_multiplier=-1)

                    def av_mms(j=j, di=di, pTt=pTt):
                        # one group for the whole stripe bank: start only on
                        # the very first matmul, stop on the very last
                        vau = vaug[u][:, j, :]
                        if di < 0:
                            nc.tensor.matmul(avt[:], vau, pTt[:],
                                             start=(j == 0), stop=False,
                                             skip_group_check=True)
                        else:
                            for c in range(di, 4):
                                nc.tensor.matmul(
                                    avt[:, c * 128:(c + 1) * 128], vau,
                                    pTt[:, c * 128:(c + 1) * 128],
                                    start=(s4 == 0 and di == 0 and c == 0),
                                    stop=(di == 3 and c == 3),
                                    skip_group_check=True)
                    pend.append(av_mms)
                    flush(3)
                    pull({0: 2, 1: 2, 2: 3, 3: 4}[u])
                flush(0)
                rD = sb.tile([64, QB], F32, name=f"rD{u}_{s4}", tag="rD",
                             bufs=4)
                nc.vector.reciprocal(rD[:], avt[64:128, :])
                nc.vector.tensor_mul(outT[hr:hr + 64, qc:qc + QB],
                                     avt[0:64, :], rD[:])

            # ---- program emission ----
            emit_setup_dmas()
            for _ in gen_qkv(0):
                pass
            for _ in gen_qkv(1):
                pass
            fillers.extend(gen_qkv(s) for s in range(2, NB))
            for s4 in range(4):
                emit_attn_block(0, s4)
                emit_attn_block(1, s4)
                fillers.extend(gen_outproj(0, mm)
                               for mm in range(s4 * 4, (s4 + 1) * 4))
            for s4 in range(4):
                emit_attn_block(2, s4)
                emit_attn_block(3, s4)
                fillers.extend(gen_outproj(1, mm)
                               for mm in range(s4 * 4, (s4 + 1) * 4))
            # drain remaining fillers (tail of out-proj)
            while fillers:
                try:
                    next(fillers[0])
                except StopIteration:
                    fillers.pop(0)

    nc.compile()
    return nc


def prep_in_maps(x, rope_freqs, w_qkv, w_out):
    x = np.ascontiguousarray(x, dtype=np.float32)
    w_qkv = np.ascontiguousarray(w_qkv, dtype=np.float32)
    w_out = np.ascontiguousarray(w_out, dtype=np.float32)
    ang = np.asarray(rope_freqs, dtype=np.float64)
    cosT = np.cos(ang).T.astype(np.float32)           # [32, T]
    sinT = np.sin(ang).T.astype(np.float32)
    cosF = np.ascontiguousarray(np.tile(cosT, (4, 1))).astype(
        ml_dtypes.bfloat16)                                      # [128, T]
    sinF = np.ascontiguousarray(
        np.concatenate([sinT, -sinT, sinT, -sinT], axis=0)).astype(
        ml_dtypes.bfloat16)                                      # [128, T]

    def lay3(mat_t):  # [D, n] -> [128, CPJ, n] (c-within-chunk, chunk, n)
        n = mat_t.shape[1]
        return np.ascontiguousarray(
            mat_t.reshape(CPJ, 128, n).transpose(1, 0, 2)
        ).astype(ml_dtypes.bfloat16)

    xT3 = lay3(x.reshape(TOK, D).T)                   # [128, CPJ, TOK]

    perm64 = np.concatenate([np.arange(0, DH, 2), np.arange(1, DH, 2)])
    in_maps = []
    for c in range(NCORES):
        h0 = 2 * c
        qk_rows = np.concatenate([h0 * DH + perm64, (h0 + 1) * DH + perm64])
        v_rows = np.arange(h0 * DH, h0 * DH + 2 * DH)
        in_maps.append({
            "xT3": xT3,
            "wq3": lay3(np.ascontiguousarray(w_qkv[qk_rows, :].T)),
            "wk3": lay3(np.ascontiguousarray(w_qkv[D + qk_rows, :].T)),
            "wv3": lay3(np.ascontiguousarray(w_qkv[2 * D + v_rows, :].T)),
            "woT": np.ascontiguousarray(w_out[:, v_rows].T).astype(
                ml_dtypes.bfloat16),
            "cosF": cosF, "sinF": sinF,
        })
    return in_maps


_CACHED = {}


def kernel(x, rope_freqs, w_qkv, w_out):
    from concourse.bass_utils import run_bass_kernel_spmd
    if "nc" not in _CACHED:
        _CACHED["nc"] = build_program()
    nc = _CACHED["nc"]
    in_maps = prep_in_maps(x, rope_freqs, w_qkv, w_out)
    res = run_bass_kernel_spmd(nc, in_maps, list(range(NCORES)))
    acc = np.zeros((TOK, D), dtype=np.float32)
    for r in res.results:
        acc += np.asarray(r["partial"], dtype=np.float32)
    return acc.reshape(B, T, D)


# revision 71
# speedup vs baseline: 1.0156x; 1.0156x over previous
"""Causal self-attention TRN2 kernel: 8-way head-parallel (2 heads/core).

Layout strategy (per core c, heads h0=2c, h1=2c+1):
  - Host pre-lays x -> xT3 [128, 8, 4096] bf16 (c-within-chunk, c-chunk, token)
    and weights [128, 8, 128] bf16 with q/k head dims [32 evens | 32 odds]
    (de-interleaved RoPE); cos/sin tables pre-broadcast to [128, T] bf16 with
    the RoPE sign pattern baked into sin rows.
  - QKV: q/k computed transposed ([head-dim, tok]) via bf16 matmuls over 8
    contraction chunks; V computed directly in [tok, head-dim] layout (lhsT =
    x chunk), killing the PE transposes the AV matmul would otherwise need.
  - RoPE on q/k PSUM with partition-aligned DVE ops (signed sin table +
    32-row swap via SBUF-SBUF DMA); result written bf16 into qkT.
  - V_aug [tok, V(64) | ones(64)] bf16 so the AV matmul also produces the
    softmax denominator in psum rows 64:127.
  - Scores computed TRANSPOSED: S^T[k,q] per 128-k-chunk; exp on ACT
    (scale=1/8, no max subtraction; |s|<~20); causal diagonal handled at
    128-col granularity: columns left of the diagonal chunk are never
    computed, only the 128x128 triangle piece gets a gpsimd affine_select.
  - AV accumulates per 128-col piece so unmasked pieces never wait on the
    mask; out-proj: lhsT=outT chunk [128,128] bf16, rhs=woT [128,1024] bf16.
  - Emission interleaves QKV/out-proj matmuls as fillers between attention
    chunks so PE fills the gaps while ACT runs exp.
  - Host sums 8 bf16 partials (the tensor-parallel all-reduce) and reshapes.
"""

import sys

if "/opt/trn_rl_repo" not in sys.path:
    sys.path.insert(0, "/opt/trn_rl_repo")

import numpy as np
import ml_dtypes

import concourse.bass as bass
import concourse.tile as tile
from concourse import bacc, mybir

F32 = mybir.dt.float32
BF16 = mybir.dt.bfloat16
EXP = mybir.ActivationFunctionType.Exp

B, T, D, H, DH = 2, 2048, 1024, 16, 64
NCORES = 8
TOK = B * T          # 4096
PB = 512             # qkv block (tokens)
NB = TOK // PB       # 8 blocks
QB = 512             # attention q-block
KC = 128             # k chunk
CPJ = D // 128       # 8 contraction chunks
FILL_RATE = 3        # filler PE-ops pulled per attention chunk


def build_program():
    nc = bacc.Bacc("TRN2", target_bir_lowering=False, debug=False,
                   num_devices=NCORES)
    xT3 = nc.dram_tensor("xT3", [128, CPJ, TOK], BF16, kind="ExternalInput").ap()
    wq3 = nc.dram_tensor("wq3", [128, CPJ, 128], BF16, kind="ExternalInput").ap()
    wk3 = nc.dram_tensor("wk3", [128, CPJ, 128], BF16, kind="ExternalInput").ap()
    wv3 = nc.dram_tensor("wv3", [128, CPJ, 128], BF16, kind="ExternalInput").ap()
    woT = nc.dram_tensor("woT", [128, D], BF16, kind="ExternalInput").ap()
    cosF = nc.dram_tensor("cosF", [128, T], BF16, kind="ExternalInput").ap()
    sinF = nc.dram_tensor("sinF", [128, T], BF16, kind="ExternalInput").ap()
    partial = nc.dram_tensor("partial", [TOK, D], BF16,
                             kind="ExternalOutput").ap()

    with tile.TileContext(nc) as tc:
        with tc.tile_pool(name="sb", bufs=1) as sb, \
             tc.tile_pool(name="ps", bufs=1, space="PSUM") as ps:
            # persistent SBUF tiles
            xfull = sb.tile([128, CPJ, TOK], BF16, name="xfull", tag="xfull")
            wq_sb = sb.tile([128, CPJ, 128], BF16, name="wq_sb", tag="wq_sb")
            wk_sb = sb.tile([128, CPJ, 128], BF16, name="wk_sb", tag="wk_sb")
            wv_sb = sb.tile([128, CPJ, 128], BF16, name="wv_sb", tag="wv_sb")
            woT_sb = sb.tile([128, D], BF16, name="woT_sb", tag="woT_sb")
            cos_full = sb.tile([128, T], BF16, name="cos_full", tag="cos_full")
            sin_full = sb.tile([128, T], BF16, name="sin_full", tag="sin_full")
            qkT = sb.tile([128, 2, TOK], BF16, name="qkT", tag="qkT")
            outT = sb.tile([128, TOK], BF16, name="outT", tag="outT")
            vaug = [sb.tile([128, T // KC, 128], BF16, name=f"vaug{u}",
                            tag=f"vaug{u}") for u in range(4)]

            warm = sb.tile([128, 128], BF16, name="warm", tag="warm")

            def emit_setup_dmas():
                nc.gpsimd.memset(warm[:], 0.0)
                nc.sync.dma_start(out=xfull[:, :, 0:PB], in_=xT3[:, :, 0:PB])
                nc.sync.dma_start(out=wq_sb[:], in_=wq3[:])
                nc.sync.dma_start(out=wk_sb[:], in_=wk3[:])
                nc.sync.dma_start(out=cos_full[:], in_=cosF[:])
                nc.sync.dma_start(out=sin_full[:], in_=sinF[:])
                nc.sync.dma_start(out=wv_sb[:], in_=wv3[:])
                nc.sync.dma_start(out=xfull[:, :, PB:2 * PB],
                                  in_=xT3[:, :, PB:2 * PB])
                nc.sync.dma_start(out=woT_sb[:], in_=woT[:])
                for u in range(4):
                    nc.gpsimd.memset(vaug[u][:, :, 64:128], 1.0)
                # PE warm-up during the DMA lead-in: ramps the tensor engine
                # to full p-state before the first real matmul.
                for i in range(48):
                    wp = ps.tile([128, 512], F32, name=f"warm{i}", tag="ops",
                                 bufs=2)
                    nc.tensor.matmul(wp[:, 0:128], warm[:], warm[:],
                                     start=True, stop=True)

            block_done = [False] * NB

            def gen_qkv(s):
                """QKV block s (tokens s*PB..): yields after each PE matmul."""
                scol = s * PB
                tcol = (s % (T // PB)) * PB
                b = s // (T // PB)
                if s + 2 < NB:   # just-in-time load of a later x slice
                    nxt = (s + 2) * PB
                    nc.sync.dma_start(out=xfull[:, :, nxt:nxt + PB],
                                      in_=xT3[:, :, nxt:nxt + PB])
                qp = ps.tile([128, PB], F32, name=f"qps{s}", tag="qps", bufs=1)
                kp = ps.tile([128, PB], F32, name=f"kps{s}", tag="kps", bufs=1)
                vp = ps.tile([128, 4, 128], F32, name=f"vps{s}", tag="ops",
                             bufs=2)
                for j in range(CPJ):
                    st, sp = (j == 0), (j == CPJ - 1)
                    nc.tensor.matmul(qp[:], wq_sb[:, j, :],
                                     xfull[:, j, scol:scol + PB],
                                     start=st, stop=sp)
                    yield
                    nc.tensor.matmul(kp[:], wk_sb[:, j, :],
                                     xfull[:, j, scol:scol + PB],
                                     start=st, stop=sp)
                    yield
                # RoPE: ra = raw*cos, rs = raw*sin, rw = 32-row swap of rs
                ra = sb.tile([128, 2, PB], BF16, name=f"ra{s}", tag="ra",
                             bufs=2)
                rs = sb.tile([128, 2, PB], BF16, name=f"rs{s}", tag="rs",
                             bufs=2)
                rw = sb.tile([128, 2, PB], BF16, name=f"rw{s}", tag="rw",
                             bufs=2)
                cs = cos_full[:, tcol:tcol + PB]
                sn = sin_full[:, tcol:tcol + PB]
                nc.vector.tensor_mul(ra[:, 0, :], qp[:], cs)
                nc.vector.tensor_mul(rs[:, 0, :], qp[:], sn)
                nc.vector.tensor_mul(ra[:, 1, :], kp[:], cs)
                nc.vector.tensor_mul(rs[:, 1, :], kp[:], sn)
                for blk in range(4):
                    src = (blk ^ 1) * 32
                    nc.sync.dma_start(out=rw[blk * 32:(blk + 1) * 32, :, :],
                                      in_=rs[src:src + 32, :, :])
                nc.vector.tensor_add(qkT[:, 0, scol:scol + PB], ra[:, 0, :],
                                     rw[:, 0, :])
                nc.vector.tensor_add(qkT[:, 1, scol:scol + PB], ra[:, 1, :],
                                     rw[:, 1, :])
                for j in range(CPJ):
                    for tt in range(4):
                        # one accumulation group per psum bank: single
                        # start on the first matmul, stop on the last
                        # (first touch of each byte overwrites pending-zero)
                        nc.tensor.matmul(
                            vp[:, tt, :],
                            xfull[:, j, scol + tt * 128:scol + (tt + 1) * 128],
                            wv_sb[:, j, :], start=(j == 0 and tt == 0),
                            stop=(j == CPJ - 1 and tt == 3),
                            skip_group_check=True)
                        yield
                ck0 = (s % (T // PB)) * 4
                for h in range(2):
                    nc.vector.tensor_copy(
                        vaug[b * 2 + h][:, ck0:ck0 + 4, 0:64],
                        vp[:, :, h * 64:(h + 1) * 64])
                block_done[s] = True

            # ---- filler management ----
            fillers = []

            def pull(n):
                for _ in range(n):
                    while fillers:
                        try:
                            next(fillers[0])
                            break
                        except StopIteration:
                            fillers.pop(0)
                    else:
                        return

            def need_block(sbi):
                while not block_done[sbi]:
                    if not fillers:
                        raise RuntimeError(f"no fillers but block {sbi} undone")
                    try:
                        next(fillers[0])
                    except StopIteration:
                        fillers.pop(0)

            def gen_outproj(bt, mm):
                col = bt * T + mm * 128
                ob = sb.tile([128, D], BF16, name=f"osb{bt}_{mm}", tag="osb",
                             bufs=8)
                opt0 = ps.tile([128, 512], F32, name=f"opsa{bt}_{mm}",
                               tag="ops", bufs=2)
                nc.tensor.matmul(opt0[:], outT[:, col:col + 128],
                                 woT_sb[:, 0:512], start=True, stop=True)
                yield
                opt1 = ps.tile([128, 512], F32, name=f"opsb{bt}_{mm}",
                               tag="ops", bufs=2)
                if mm % 2 == 1 or mm >= 12:
                    nc.scalar.copy(out=ob[:, 0:512], in_=opt0[:])
                else:
                    nc.vector.tensor_copy(out=ob[:, 0:512], in_=opt0[:])
                nc.tensor.matmul(opt1[:], outT[:, col:col + 128],
                                 woT_sb[:, 512:1024], start=True, stop=True)
                yield
                nc.vector.tensor_copy(out=ob[:, 512:1024], in_=opt1[:])
                nc.sync.dma_start(out=partial[col:col + 128, :], in_=ob[:])

            def emit_attn_block(u, s4):
                """Attention for unit u (batch u//2, head u%2), q-block s4."""
                b, h = u // 2, u % 2
                hr = h * 64
                tb = b * T
                qc = tb + s4 * QB
                need_block(b * 4 + s4)
                avt = ps.tile([128, QB], F32, name=f"av{u}_{s4}", tag="av",
                              bufs=1)
                njc = (s4 + 1) * 4
                pend = []           # deferred av matmuls (stagger depth 2)

                def flush(k):
                    while len(pend) > k:
                        pend.pop(0)()

                for j in range(njc):
                    di = j - s4 * 4          # >=0: diagonal chunk index
                    c0 = di * 128 if di > 0 else 0
                    kc = tb + j * KC
                    spst = ps.tile([128, QB], F32, name=f"sps{u}_{s4}_{j}",
                                   tag="sps", bufs=3)
                    nc.tensor.matmul(spst[:, c0:QB],
                                     qkT[hr:hr + 64, 1, kc:kc + KC],
                                     qkT[hr:hr + 64, 0, qc + c0:qc + QB],
                                     start=True, stop=True)
                    pTt = sb.tile([128, QB], BF16, name=f"pT{u}_{s4}_{j}",
                                  tag="pT", bufs=10)
                    nc.scalar.activation(pTt[:, c0:QB], spst[:, c0:QB], EXP,
                                         scale=0.125)
                    if di >= 0:
                        nc.gpsimd.affine_select(
                            out=pTt[:, di * 128:(di + 1) * 128],
                            in_=pTt[:, di * 128:(di + 1) * 128],
                            compare_op=mybir.AluOpType.is_ge,
                            fill=0.0, base=0, pattern=[[1, 128]],
                            channel_multiplier=-1)

                    def av_mms(j=j, di=di, pTt=pTt):
                        # one group for the whole stripe bank: start only on
                        # the very first matmul, stop on the very last
                        vau = vaug[u][:, j, :]
                        if di < 0:
                            nc.tensor.matmul(avt[:], vau, pTt[:],
                                             start=(j == 0), stop=False,
                                             skip_group_check=True)
                        else:
                            for c in range(di, 4):
                                nc.tensor.matmul(
                                    avt[:, c * 128:(c + 1) * 128], vau,
                                    pTt[:, c * 128:(c + 1) * 128],
                                    start=(s4 == 0 and di == 0 and c == 0),
                                    stop=(di == 3 and c == 3),
                                    skip_group_check=True)
                    pend.append(av_mms)
                    flush(3)
                    pull({0: 2, 1: 2, 2: 3, 3: 4}[u])
                flush(0)
                rD = sb.tile([64, QB], F32, name=f"rD{u}_{s4}", tag="rD",
                             bufs=4)
                nc.vector.reciprocal(rD[:], avt[64:128, :])
                nc.vector.tensor_mul(outT[hr:hr + 64, qc:qc + QB],
                                     avt[0:64, :], rD[:])

            # ---- program emission ----
            emit_setup_dmas()
            for _ in gen_qkv(0):
                pass
            for _ in gen_qkv(1):
                pass
            fillers.extend(gen_qkv(s) for s in range(2, NB))
            for s4 in range(4):
                emit_attn_block(0, s4)
                emit_attn_block(1, s4)
                fillers.extend(gen_outproj(0, mm)
                               for mm in range(s4 * 4, (s4 + 1) * 4))
            for s4 in range(4):
                emit_attn_block(2, s4)
                emit_attn_block(3, s4)
                fillers.extend(gen_outproj(1, mm)
                               for mm in range(s4 * 4, (s4 + 1) * 4))
            # drain remaining fillers (tail of out-proj)
            while fillers:
                try:
                    next(fillers[0])
                except StopIteration:
                    fillers.pop(0)

    nc.compile()
    return nc


def prep_in_maps(x, rope_freqs, w_qkv, w_out):
    x = np.ascontiguousarray(x, dtype=np.float32)
    w_qkv = np.ascontiguousarray(w_qkv, dtype=np.float32)
    w_out = np.ascontiguousarray(w_out, dtype=np.float32)
    ang = np.asarray(rope_freqs, dtype=np.float64)
    cosT = np.cos(ang).T.astype(np.float32)           # [32, T]
    sinT = np.sin(ang).T.astype(np.float32)
    cosF = np.ascontiguousarray(np.tile(cosT, (4, 1))).astype(
        ml_dtypes.bfloat16)                                      # [128, T]
    sinF = np.ascontiguousarray(
        np.concatenate([sinT, -sinT, sinT, -sinT], axis=0)).astype(
        ml_dtypes.bfloat16)                                      # [128, T]

    def lay3(mat_t):  # [D, n] -> [128, CPJ, n] (c-within-chunk, chunk, n)
        n = mat_t.shape[1]
        return np.ascontiguousarray(
            mat_t.reshape(CPJ, 128, n).transpose(1, 0, 2)
        ).astype(ml_dtypes.bfloat16)

    xT3 = lay3(x.reshape(TOK, D).T)                   # [128, CPJ, TOK]

    perm64 = np.concatenate([np.arange(0, DH, 2), np.arange(1, DH, 2)])
    in_maps = []
    for c in range(NCORES):
        h0 = 2 * c
        qk_rows = np.concatenate([h0 * DH + perm64, (h0 + 1) * DH + perm64])
        v_rows = np.arange(h0 * DH, h0 * DH + 2 * DH)
        in_maps.append({
            "xT3": xT3,
            "wq3": lay3(np.ascontiguousarray(w_qkv[qk_rows, :].T)),
            "wk3": lay3(np.ascontiguousarray(w_qkv[D + qk_rows, :].T)),
            "wv3": lay3(np.ascontiguousarray(w_qkv[2 * D + v_rows, :].T)),
            "woT": np.ascontiguousarray(w_out[:, v_rows].T).astype(
                ml_dtypes.bfloat16),
            "cosF": cosF, "sinF": sinF,
        })
    return in_maps


_CACHED = {}


def kernel(x, rope_freqs, w_qkv, w_out):
    from concourse.bass_utils import run_bass_kernel_spmd
    if "nc" not in _CACHED:
        _CACHED["nc"] = build_program()
    nc = _CACHED["nc"]
    in_maps = prep_in_maps(x, rope_freqs, w_qkv, w_out)
    res = run_bass_kernel_spmd(nc, in_maps, list(range(NCORES)))
    acc = np.zeros((TOK, D), dtype=np.float32)
    for r in res.results:
        acc += np.asarray(r["partial"], dtype=np.float32)
    return acc.reshape(B, T, D)


# revision 80
# speedup vs baseline: 1.0304x; 1.0146x over previous
"""Causal self-attention TRN2 kernel: 8-way head-parallel (2 heads/core).

Layout strategy (per core c, heads h0=2c, h1=2c+1):
  - Host pre-lays x -> xT3 [128, 8, 4096] bf16 (c-within-chunk, c-chunk, token)
    and weights [128, 8, 128] bf16 with q/k head dims [32 evens | 32 odds]
    (de-interleaved RoPE); cos/sin tables pre-broadcast to [128, T] bf16 with
    the RoPE sign pattern baked into sin rows.
  - QKV: q/k computed transposed ([head-dim, tok]) via bf16 matmuls over 8
    contraction chunks; V computed directly in [tok, head-dim] layout (lhsT =
    x chunk), killing the PE transposes the AV matmul would otherwise need.
  - RoPE on q/k PSUM with partition-aligned DVE ops (signed sin table +
    32-row swap via SBUF-SBUF DMA); result written bf16 into qkT.
  - V_aug [tok, V(64) | ones(64)] bf16 so the AV matmul also produces the
    softmax denominator in psum rows 64:127.
  - Scores computed TRANSPOSED: S^T[k,q] per 128-k-chunk; exp on ACT
    (scale=1/8, no max subtraction; |s|<~20); causal diagonal handled at
    128-col granularity: columns left of the diagonal chunk are never
    computed, only the 128x128 triangle piece gets a gpsimd affine_select.
  - AV accumulates per 128-col piece so unmasked pieces never wait on the
    mask; out-proj: lhsT=outT chunk [128,128] bf16, rhs=woT [128,1024] bf16.
  - Emission interleaves QKV/out-proj matmuls as fillers between attention
    chunks so PE fills the gaps while ACT runs exp.
  - Host sums 8 bf16 partials (the tensor-parallel all-reduce) and reshapes.
"""

import sys

if "/opt/trn_rl_repo" not in sys.path:
    sys.path.insert(0, "/opt/trn_rl_repo")

import numpy as np
import ml_dtypes

import concourse.bass as bass
import concourse.tile as tile
from concourse import bacc, mybir

F32 = mybir.dt.float32
BF16 = mybir.dt.bfloat16
EXP = mybir.ActivationFunctionType.Exp

B, T, D, H, DH = 2, 2048, 1024, 16, 64
NCORES = 8
TOK = B * T          # 4096
PB = 512             # qkv block (tokens)
NB = TOK // PB       # 8 blocks
QB = 512             # attention q-block
KC = 128             # k chunk
CPJ = D // 128       # 8 contraction chunks
FILL_RATE = 3        # filler PE-ops pulled per attention chunk


def build_program():
    nc = bacc.Bacc("TRN2", target_bir_lowering=False, debug=False,
                   num_devices=NCORES)
    xT3 = nc.dram_tensor("xT3", [128, CPJ, TOK], BF16, kind="ExternalInput").ap()
    wq3 = nc.dram_tensor("wq3", [128, CPJ, 128], BF16, kind="ExternalInput").ap()
    wk3 = nc.dram_tensor("wk3", [128, CPJ, 128], BF16, kind="ExternalInput").ap()
    wv3 = nc.dram_tensor("wv3", [128, CPJ, 128], BF16, kind="ExternalInput").ap()
    woT = nc.dram_tensor("woT", [128, D], BF16, kind="ExternalInput").ap()
    cosF = nc.dram_tensor("cosF", [128, T], BF16, kind="ExternalInput").ap()
    sinF = nc.dram_tensor("sinF", [128, T], BF16, kind="ExternalInput").ap()
    partial = nc.dram_tensor("partial", [TOK, D], BF16,
                             kind="ExternalOutput").ap()

    with tile.TileContext(nc) as tc:
        with tc.tile_pool(name="sb", bufs=1) as sb, \
             tc.tile_pool(name="ps", bufs=1, space="PSUM") as ps:
            # persistent SBUF tiles
            xfull = sb.tile([128, CPJ, TOK], BF16, name="xfull", tag="xfull")
            wq_sb = sb.tile([128, CPJ, 128], BF16, name="wq_sb", tag="wq_sb")
            wk_sb = sb.tile([128, CPJ, 128], BF16, name="wk_sb", tag="wk_sb")
            wv_sb = sb.tile([128, CPJ, 128], BF16, name="wv_sb", tag="wv_sb")
            woT_sb = sb.tile([128, D], BF16, name="woT_sb", tag="woT_sb")
            cos_full = sb.tile([128, T], BF16, name="cos_full", tag="cos_full")
            sin_full = sb.tile([128, T], BF16, name="sin_full", tag="sin_full")
            qkT = sb.tile([128, 2, TOK], BF16, name="qkT", tag="qkT")
            outT = sb.tile([128, TOK], BF16, name="outT", tag="outT")
            vaug = [sb.tile([128, T // KC, 128], BF16, name=f"vaug{u}",
                            tag=f"vaug{u}") for u in range(4)]

            warm = sb.tile([128, 128], BF16, name="warm", tag="warm")

            def emit_setup_dmas():
                nc.gpsimd.memset(warm[:], 0.0)
                nc.sync.dma_start(out=xfull[:, :, 0:PB], in_=xT3[:, :, 0:PB])
                nc.sync.dma_start(out=wq_sb[:], in_=wq3[:])
                nc.sync.dma_start(out=wk_sb[:], in_=wk3[:])
                nc.sync.dma_start(out=cos_full[:], in_=cosF[:])
                nc.sync.dma_start(out=sin_full[:], in_=sinF[:])
                nc.sync.dma_start(out=wv_sb[:], in_=wv3[:])
                nc.sync.dma_start(out=xfull[:, :, PB:2 * PB],
                                  in_=xT3[:, :, PB:2 * PB])
                nc.sync.dma_start(out=woT_sb[:], in_=woT[:])
                for u in range(4):
                    nc.gpsimd.memset(vaug[u][:, :, 64:128], 1.0)
                # PE warm-up during the DMA lead-in: ramps the tensor engine
                # to full p-state before the first real matmul.
                for i in range(48):
                    wp = ps.tile([128, 512], F32, name=f"warm{i}", tag="ops",
                                 bufs=2)
                    nc.tensor.matmul(wp[:, 0:128], warm[:], warm[:],
                                     start=True, stop=True)

            block_done = [False] * NB

            def gen_qkv(s):
                """QKV block s (tokens s*PB..): yields after each PE matmul."""
                scol = s * PB
                tcol = (s % (T // PB)) * PB
                b = s // (T // PB)
                if s + 2 < NB:   # just-in-time load of a later x slice
                    nxt = (s + 2) * PB
                    nc.sync.dma_start(out=xfull[:, :, nxt:nxt + PB],
                                      in_=xT3[:, :, nxt:nxt + PB])
                qp = ps.tile([128, PB], F32, name=f"qps{s}", tag="qps", bufs=1)
                kp = ps.tile([128, PB], F32, name=f"kps{s}", tag="kps", bufs=1)
                vp = ps.tile([128, 4, 128], F32, name=f"vps{s}", tag="ops",
                             bufs=2)
                for j in range(CPJ):
                    st, sp = (j == 0), (j == CPJ - 1)
                    nc.tensor.matmul(qp[:], wq_sb[:, j, :],
                                     xfull[:, j, scol:scol + PB],
                                     start=st, stop=sp)
                    yield
                    nc.tensor.matmul(kp[:], wk_sb[:, j, :],
                                     xfull[:, j, scol:scol + PB],
                                     start=st, stop=sp)
                    yield
                # RoPE: ra = raw*cos, rs = raw*sin, rw = 32-row swap of rs
                ra = sb.tile([128, 2, PB], BF16, name=f"ra{s}", tag="ra",
                             bufs=2)
                rs = sb.tile([128, 2, PB], BF16, name=f"rs{s}", tag="rs",
                             bufs=2)
                rw = sb.tile([128, 2, PB], BF16, name=f"rw{s}", tag="rw",
                             bufs=2)
                cs = cos_full[:, tcol:tcol + PB]
                sn = sin_full[:, tcol:tcol + PB]
                nc.vector.tensor_mul(ra[:, 0, :], qp[:], cs)
                nc.vector.tensor_mul(rs[:, 0, :], qp[:], sn)
                nc.vector.tensor_mul(ra[:, 1, :], kp[:], cs)
                nc.vector.tensor_mul(rs[:, 1, :], kp[:], sn)
                for blk in range(4):
                    src = (blk ^ 1) * 32
                    nc.sync.dma_start(out=rw[blk * 32:(blk + 1) * 32, :, :],
                                      in_=rs[src:src + 32, :, :])
                nc.vector.tensor_add(qkT[:, 0, scol:scol + PB], ra[:, 0, :],
                                     rw[:, 0, :])
                nc.vector.tensor_add(qkT[:, 1, scol:scol + PB], ra[:, 1, :],
                                     rw[:, 1, :])
                for j in range(CPJ):
                    for tt in range(4):
                        # one accumulation group per psum bank: single
                        # start on the first matmul, stop on the last
                        # (first touch of each byte overwrites pending-zero)
                        nc.tensor.matmul(
                            vp[:, tt, :],
                            xfull[:, j, scol + tt * 128:scol + (tt + 1) * 128],
                            wv_sb[:, j, :], start=(j == 0 and tt == 0),
                            stop=(j == CPJ - 1 and tt == 3),
                            skip_group_check=True)
                        yield
                ck0 = (s % (T // PB)) * 4
                for h in range(2):
                    nc.vector.tensor_copy(
                        vaug[b * 2 + h][:, ck0:ck0 + 4, 0:64],
                        vp[:, :, h * 64:(h + 1) * 64])
                block_done[s] = True

            # ---- filler management ----
            fillers = []

            def pull(n):
                for _ in range(n):
                    while fillers:
                        try:
                            next(fillers[0])
                            break
                        except StopIteration:
                            fillers.pop(0)
                    else:
                        return

            def need_block(sbi):
                while not block_done[sbi]:
                    if not fillers:
                        raise RuntimeError(f"no fillers but block {sbi} undone")
                    try:
                        next(fillers[0])
                    except StopIteration:
                        fillers.pop(0)

            def gen_outproj(bt, mm):
                col = bt * T + mm * 128
                ob = sb.tile([128, D], BF16, name=f"osb{bt}_{mm}", tag="osb",
                             bufs=8)
                # batch1: odd tiles borrow the retired qps/kps banks,
                # widening the out-proj psum pipeline to 4 slots
                brw = bt == 1 and mm % 2 == 1
                opt0 = ps.tile([128, 512], F32, name=f"opsa{bt}_{mm}",
                               tag="qps" if brw else "ops",
                               bufs=1 if brw else 2)
                nc.tensor.matmul(opt0[:], outT[:, col:col + 128],
                                 woT_sb[:, 0:512], start=True, stop=True)
                yield
                opt1 = ps.tile([128, 512], F32, name=f"opsb{bt}_{mm}",
                               tag="kps" if brw else "ops",
                               bufs=1 if brw else 2)
                if mm % 2 == 1 or mm >= 12:
                    nc.scalar.copy(out=ob[:, 0:512], in_=opt0[:])
                else:
                    nc.vector.tensor_copy(out=ob[:, 0:512], in_=opt0[:])
                nc.tensor.matmul(opt1[:], outT[:, col:col + 128],
                                 woT_sb[:, 512:1024], start=True, stop=True)
                yield
                nc.vector.tensor_copy(out=ob[:, 512:1024], in_=opt1[:])
                nc.sync.dma_start(out=partial[col:col + 128, :], in_=ob[:])

            def emit_attn_block(u, s4):
                """Attention for unit u (batch u//2, head u%2), q-block s4."""
                b, h = u // 2, u % 2
                hr = h * 64
                tb = b * T
                qc = tb + s4 * QB
                need_block(b * 4 + s4)
                avt = ps.tile([128, QB], F32, name=f"av{u}_{s4}", tag="av",
                              bufs=1)
                njc = (s4 + 1) * 4
                pend = []           # deferred av matmuls (stagger depth 2)

                def flush(k):
                    while len(pend) > k:
                        pend.pop(0)()

                for j in range(njc):
                    di = j - s4 * 4          # >=0: diagonal chunk index
                    c0 = di * 128 if di > 0 else 0
                    kc = tb + j * KC
                    spst = ps.tile([128, QB], F32, name=f"sps{u}_{s4}_{j}",
                                   tag="sps", bufs=3)
                    nc.tensor.matmul(spst[:, c0:QB],
                                     qkT[hr:hr + 64, 1, kc:kc + KC],
                                     qkT[hr:hr + 64, 0, qc + c0:qc + QB],
                                     start=True, stop=True)
                    pTt = sb.tile([128, QB], BF16, name=f"pT{u}_{s4}_{j}",
                                  tag="pT", bufs=10)
                    nc.scalar.activation(pTt[:, c0:QB], spst[:, c0:QB], EXP,
                                         scale=0.125)
                    if di >= 0:
                        nc.gpsimd.affine_select(
                            out=pTt[:, di * 128:(di + 1) * 128],
                            in_=pTt[:, di * 128:(di + 1) * 128],
                            compare_op=mybir.AluOpType.is_ge,
                            fill=0.0, base=0, pattern=[[1, 128]],
                            channel_multiplier=-1)

                    def av_mms(j=j, di=di, pTt=pTt):
                        # one group for the whole stripe bank: start only on
                        # the very first matmul, stop on the very last
                        vau = vaug[u][:, j, :]
                        if di < 0:
                            nc.tensor.matmul(avt[:], vau, pTt[:],
                                             start=(j == 0), stop=False,
                                             skip_group_check=True)
                        else:
                            for c in range(di, 4):
                                nc.tensor.matmul(
                                    avt[:, c * 128:(c + 1) * 128], vau,
                                    pTt[:, c * 128:(c + 1) * 128],
                                    start=(s4 == 0 and di == 0 and c == 0),
                                    stop=(di == 3 and c == 3),
                                    skip_group_check=True)
                    pend.append(av_mms)
                    flush(3)
                    pull({0: 2, 1: 2, 2: 3, 3: 6}[u])
                flush(0)
                rD = sb.tile([64, QB], F32, name=f"rD{u}_{s4}", tag="rD",
                             bufs=4)
                nc.vector.reciprocal(rD[:], avt[64:128, :])
                nc.vector.tensor_mul(outT[hr:hr + 64, qc:qc + QB],
                                     avt[0:64, :], rD[:])

            # ---- program emission ----
            emit_setup_dmas()
            for _ in gen_qkv(0):
                pass
            for _ in gen_qkv(1):
                pass
            fillers.extend(gen_qkv(s) for s in range(2, NB))
            for s4 in range(4):
                emit_attn_block(0, s4)
                emit_attn_block(1, s4)
                fillers.extend(gen_outproj(0, mm)
                               for mm in range(s4 * 4, (s4 + 1) * 4))
            for s4 in range(4):
                emit_attn_block(2, s4)
                emit_attn_block(3, s4)
                fillers.extend(gen_outproj(1, mm)
                               for mm in range(s4 * 4, (s4 + 1) * 4))
            # drain remaining fillers (tail of out-proj)
            while fillers:
                try:
                    next(fillers[0])
                except StopIteration:
                    fillers.pop(0)

    nc.compile()
    return nc


def prep_in_maps(x, rope_freqs, w_qkv, w_out):
    x = np.ascontiguousarray(x, dtype=np.float32)
    w_qkv = np.ascontiguousarray(w_qkv, dtype=np.float32)
    w_out = np.ascontiguousarray(w_out, dtype=np.float32)
    ang = np.asarray(rope_freqs, dtype=np.float64)
    cosT = np.cos(ang).T.astype(np.float32)           # [32, T]
    sinT = np.sin(ang).T.astype(np.float32)
    cosF = np.ascontiguousarray(np.tile(cosT, (4, 1))).astype(
        ml_dtypes.bfloat16)                                      # [128, T]
    sinF = np.ascontiguousarray(
        np.concatenate([sinT, -sinT, sinT, -sinT], axis=0)).astype(
        ml_dtypes.bfloat16)                                      # [128, T]

    def lay3(mat_t):  # [D, n] -> [128, CPJ, n] (c-within-chunk, chunk, n)
        n = mat_t.shape[1]
        return np.ascontiguousarray(
            mat_t.reshape(CPJ, 128, n).transpose(1, 0, 2)
        ).astype(ml_dtypes.bfloat16)

    xT3 = lay3(x.reshape(TOK, D).T)                   # [128, CPJ, TOK]

    perm64 = np.concatenate([np.arange(0, DH, 2), np.arange(1, DH, 2)])
    in_maps = []
    for c in range(NCORES):
        h0 = 2 * c
        qk_rows = np.concatenate([h0 * DH + perm64, (h0 + 1) * DH + perm64])
        v_rows = np.arange(h0 * DH, h0 * DH + 2 * DH)
        in_maps.append({
            "xT3": xT3,
            "wq3": lay3(np.ascontiguousarray(w_qkv[qk_rows, :].T)),
            "wk3": lay3(np.ascontiguousarray(w_qkv[D + qk_rows, :].T)),
            "wv3": lay3(np.ascontiguousarray(w_qkv[2 * D + v_rows, :].T)),
            "woT": np.ascontiguousarray(w_out[:, v_rows].T).astype(
                ml_dtypes.bfloat16),
            "cosF": cosF, "sinF": sinF,
        })
    return in_maps


_CACHED = {}


def kernel(x, rope_freqs, w_qkv, w_out):
    from concourse.bass_utils import run_bass_kernel_spmd
    if "nc" not in _CACHED:
        _CACHED["nc"] = build_program()
    nc = _CACHED["nc"]
    in_maps = prep_in_maps(x, rope_freqs, w_qkv, w_out)
    res = run_bass_kernel_spmd(nc, in_maps, list(range(NCORES)))
    acc = np.zeros((TOK, D), dtype=np.float32)
    for r in res.results:
        acc += np.asarray(r["partial"], dtype=np.float32)
    return acc.reshape(B, T, D)
